# revision 3
# baseline (speedup 1.0000x reference)
"""Causal self-attention (B=4, S=2048, D=1024) on 8 trn2 cores, v5.

kernel4 (interleaved K/V split, host merge of unnormalized partials) plus:
the Q^T projection is also split across the batch pair — each core projects
its q-half [1024 queries] first, the halves are exchanged with a pairwise
AllGather through DRAM bounce buffers while the K/V projections run, and
the gathered full Q^T feeds the attention. Per-core PE work ~348k cycles
(~145us @2.4GHz).
"""

import numpy as np
from contextlib import ExitStack

import concourse.bass as bass
import concourse.tile as tile
import concourse.mybir as mybir
from concourse import bacc
from concourse.bass_utils import run_bass_kernel_spmd

F32 = mybir.dt.float32
BF16 = mybir.dt.bfloat16
AFT = mybir.ActivationFunctionType
NP_BF16 = mybir.dt.np(mybir.dt.bfloat16)

B, S, D = 4, 2048, 1024
P = 128
QTILE = 256
NG = S // QTILE      # 8 query tiles (all of the batch)
DC = D // P
EC = D // P
NKO = 8              # own kc blocks per core
HQ = S // 2          # own q-half size
SB = 512
SCALE = 1.0 / np.sqrt(D)
MASK_NEG = -1.0e9
GROUPS = [[0, 1], [2, 3], [4, 5], [6, 7]]

_NC_CACHE = None


def _emit(nc, tc, ctx, xqo, xkv, wqt, wkt, wvt, msk, out, rout):
    persist = ctx.enter_context(tc.tile_pool(name="persist", bufs=1))
    dram = ctx.enter_context(tc.tile_pool(name="dram", bufs=1, space="DRAM"))

    ones2 = persist.tile([P, 2], BF16)
    nc.vector.memset(ones2[:], 1.0)

    KT = persist.tile([P, EC, NKO * P], BF16)   # K^T own: [e-part, ec, kslot*128]
    V = persist.tile([P, NKO, D], BF16)         # V own:   [k-part, kslot, e]
    QT = persist.tile([P, EC, S], BF16)         # Q^T all: [e-part, ec, q]
    mt = persist.tile([P, NG, QTILE], F32)

    qin = dram.tile([D, HQ], BF16)              # my Q^T half [e, q_own]
    qout = dram.tile([2, D, HQ], BF16)

    with tc.tile_pool(name="proj", bufs=1) as proj, \
         tc.tile_pool(name="stage", bufs=3) as stg:
        xqos = proj.tile([P, DC, HQ], BF16)     # X^T own q-half cols
        xkvs = proj.tile([P, DC, NKO * P], BF16)
        wq = proj.tile([P, DC, D], BF16)
        wk = proj.tile([P, DC, D], BF16)
        wv = proj.tile([P, DC, D], BF16)

        nc.sync.dma_start(wq[:], wqt.rearrange("(dc p) e -> p dc e", p=P))
        nc.gpsimd.dma_start(xqos[:], xqo.rearrange("(dc p) q -> p dc q", p=P))
        nc.sync.dma_start(wk[:], wkt.rearrange("(dc p) e -> p dc e", p=P))
        nc.gpsimd.dma_start(xkvs[:], xkv.rearrange("(dc p) k -> p dc k", p=P))
        nc.sync.dma_start(wv[:], wvt.rearrange("(dc p) e -> p dc e", p=P))
        nc.gpsimd.dma_start(mt[:], msk.rearrange("g p j -> p g j"))

        # ---- Q^T own half first (feeds the AllGather) ----
        with tc.tile_pool(name="qproj_ps", bufs=3, space="PSUM") as qps:
            for ec in range(EC):
                qstt = stg.tile([P, HQ], BF16, tag="qst")
                pss = [qps.tile([P, 512], F32, tag=f"pq{qh}", name=f"pq{qh}")
                       for qh in range(2)]
                for dc in range(DC):
                    for qh in range(2):
                        nc.tensor.matmul(pss[qh][:], wq[:, dc, ec * P:(ec + 1) * P],
                                         xqos[:, dc, qh * 512:(qh + 1) * 512],
                                         start=(dc == 0), stop=(dc == DC - 1))
                nc.scalar.copy(qstt[:, 0:512], pss[0][:])
                nc.vector.tensor_copy(qstt[:, 512:1024], pss[1][:])
                nc.sync.dma_start(qin[ec * P:(ec + 1) * P, :], qstt[:])
        nc.gpsimd.collective_compute(
            "AllGather", mybir.AluOpType.bypass, replica_groups=GROUPS,
            ins=[qin[:]], outs=[qout[:]])
        for r in range(2):
            for ec in range(EC):
                nc.sync.dma_start(QT[:, ec, r * HQ:(r + 1) * HQ],
                                  qout[r, ec * P:(ec + 1) * P, :])

        # ---- K^T own ----
        with tc.tile_pool(name="kproj_ps", bufs=3, space="PSUM") as kps:
            for ec in range(EC):
                pss = [kps.tile([P, SB], F32, tag=f"pk{sb}", name=f"pk{sb}")
                       for sb in range(2)]
                for dc in range(DC):
                    for sb in range(2):
                        nc.tensor.matmul(pss[sb][:], wk[:, dc, ec * P:(ec + 1) * P],
                                         xkvs[:, dc, sb * SB:(sb + 1) * SB],
                                         start=(dc == 0), stop=(dc == DC - 1))
                for sb in range(2):
                    if (ec + sb) % 2 == 0:
                        nc.scalar.copy(KT[:, ec, sb * SB:(sb + 1) * SB], pss[sb][:])
                    else:
                        nc.vector.tensor_copy(KT[:, ec, sb * SB:(sb + 1) * SB], pss[sb][:])

        # ---- V own ----
        with tc.tile_pool(name="vproj_ps", bufs=3, space="PSUM") as vps:
            for kc in range(NKO):
                pss = [vps.tile([P, 512], F32, tag=f"pv{eh}", name=f"pv{eh}")
                       for eh in range(2)]
                for dc in range(DC):
                    for eh in range(2):
                        nc.tensor.matmul(pss[eh][:], xkvs[:, dc, kc * P:(kc + 1) * P],
                                         wv[:, dc, eh * 512:(eh + 1) * 512],
                                         start=(dc == 0), stop=(dc == DC - 1))
                nc.scalar.copy(V[:, kc, 0:512], pss[0][:])
                nc.vector.tensor_copy(V[:, kc, 512:1024], pss[1][:])

    # ---------------- attention (identical to kernel4) ----------------
    with tc.tile_pool(name="attn_e", bufs=2) as pe_pool, \
         tc.tile_pool(name="attn", bufs=2) as pa, \
         tc.tile_pool(name="attn_o", bufs=4) as po, \
         tc.tile_pool(name="attn_s", bufs=3, space="PSUM") as psS, \
         tc.tile_pool(name="attn_u", bufs=2, space="PSUM") as psU, \
         tc.tile_pool(name="attn_r", bufs=1, space="PSUM") as psR:
        rt = pa.tile([P, 2 * NG], F32, tag="rt")
        for gp in range(NG // 2):
            g0 = 2 * gp
            expS = pe_pool.tile([P, NKO, 2 * QTILE], BF16, tag="expS")
            for j in range(g0 + 1):
                pS = psS.tile([P, 2 * QTILE], F32, tag="pS")
                for ec in range(EC):
                    nc.tensor.matmul(pS[:], KT[:, ec, j * P:(j + 1) * P],
                                     QT[:, ec, g0 * QTILE:(g0 + 2) * QTILE],
                                     start=(ec == 0), stop=(ec == EC - 1))
                if j == g0:
                    nc.vector.tensor_add(pS[:, 0:QTILE], pS[:, 0:QTILE], mt[:, g0, :])
                nc.scalar.activation(expS[:, j, :], pS[:], AFT.Exp, scale=SCALE)
            pSt = psS.tile([P, 2 * QTILE], F32, tag="pS")
            for ec in range(EC):
                nc.tensor.matmul(pSt[:, 0:QTILE], KT[:, ec, (g0 + 1) * P:(g0 + 2) * P],
                                 QT[:, ec, (g0 + 1) * QTILE:(g0 + 2) * QTILE],
                                 start=(ec == 0), stop=(ec == EC - 1))
            nc.vector.tensor_add(pSt[:, 0:QTILE], pSt[:, 0:QTILE], mt[:, g0 + 1, :])
            nc.scalar.activation(expS[:, g0 + 1, 256:512], pSt[:, 0:QTILE], AFT.Exp, scale=SCALE)

            for half in range(2):
                g = g0 + half
                nsl = g + 1
                for qc in range(QTILE // P):
                    pU0 = psU.tile([P, 512], F32, tag="pU0")
                    pU1 = psU.tile([P, 512], F32, tag="pU1")
                    pR = psR.tile([P, 2], F32, tag="pR")
                    for j in range(nsl):
                        lhs = expS[:, j, half * QTILE + qc * P: half * QTILE + (qc + 1) * P]
                        st, sp = (j == 0), (j == nsl - 1)
                        nc.tensor.matmul(pU0[:], lhs, V[:, j, 0:512], start=st, stop=sp)
                        nc.tensor.matmul(pU1[:], lhs, V[:, j, 512:1024], start=st, stop=sp)
                        nc.tensor.matmul(pR[:], lhs, ones2[:], start=st, stop=sp)
                    nc.vector.tensor_copy(rt[:, 2 * g + qc: 2 * g + qc + 1], pR[:, 0:1])
                    ot = po.tile([P, D], F32, tag="ot")
                    nc.scalar.copy(ot[:, 0:512], pU0[:])
                    nc.vector.tensor_copy(ot[:, 512:1024], pU1[:])
                    nc.sync.dma_start(out[(g * QTILE + qc * P):(g * QTILE + (qc + 1) * P), :], ot[:])
        nc.sync.dma_start(rout.rearrange("s p -> p s"), rt[:])


def _build(reps: int = 1):
    nc = bacc.Bacc("TRN2", target_bir_lowering=False, debug=False, num_devices=8)
    xqo = nc.dram_tensor("XqoT", [D, HQ], BF16, kind="ExternalInput").ap()
    xkv = nc.dram_tensor("XkvT", [D, NKO * P], BF16, kind="ExternalInput").ap()
    wqt = nc.dram_tensor("WqT", [D, D], BF16, kind="ExternalInput").ap()
    wkt = nc.dram_tensor("WkT", [D, D], BF16, kind="ExternalInput").ap()
    wvt = nc.dram_tensor("WvT", [D, D], BF16, kind="ExternalInput").ap()
    msk = nc.dram_tensor("Mask", [NG, P, QTILE], F32, kind="ExternalInput").ap()
    out = nc.dram_tensor("O", [S, D], F32, kind="ExternalOutput").ap()
    rout = nc.dram_tensor("R", [2 * NG, P], F32, kind="ExternalOutput").ap()

    with tile.TileContext(nc) as tc:
        for _rep in range(reps):
            with ExitStack() as ctx:
                _emit(nc, tc, ctx, xqo, xkv, wqt, wkt, wvt, msk, out, rout)

    nc.compile()
    return nc


def _get_nc():
    global _NC_CACHE
    if _NC_CACHE is None:
        _NC_CACHE = _build()
    return _NC_CACHE


def _make_masks(parity: int) -> np.ndarray:
    m = np.empty((NG, P, QTILE), dtype=np.float32)
    j = np.arange(QTILE)[None, :]
    p = np.arange(P)[:, None]
    for g in range(NG):
        kglob = (2 * g + parity) * P + p
        qglob = g * QTILE + j
        m[g] = np.where(qglob >= kglob, 0.0, MASK_NEG)
    return m


def _prep_in_maps(X, W_q, W_k, W_v):
    X = np.asarray(X, dtype=np.float32)
    WqT = np.ascontiguousarray(np.asarray(W_q, np.float32).astype(NP_BF16).T)
    WkT = np.ascontiguousarray(np.asarray(W_k, np.float32).astype(NP_BF16).T)
    WvT = np.ascontiguousarray(np.asarray(W_v, np.float32).astype(NP_BF16).T)
    Xb16 = X.astype(NP_BF16)

    masks = [_make_masks(par) for par in range(2)]
    in_maps = []
    for c in range(8):
        b, par = c // 2, c % 2
        XTb = np.ascontiguousarray(Xb16[b].T)                    # [D, S]
        kcols = np.concatenate(
            [XTb[:, (2 * j + par) * P:(2 * j + par + 1) * P]
             for j in range(NKO)], axis=1)
        in_maps.append({
            "XqoT": np.ascontiguousarray(XTb[:, par * HQ:(par + 1) * HQ]),
            "XkvT": np.ascontiguousarray(kcols),
            "WqT": WqT, "WkT": WkT, "WvT": WvT,
            "Mask": masks[par],
        })
    return in_maps


def kernel(X, W_q, W_k, W_v):
    in_maps = _prep_in_maps(X, W_q, W_k, W_v)
    global _last_in_maps
    _last_in_maps = in_maps
    nc = _get_nc()
    res = run_bass_kernel_spmd(nc, in_maps, core_ids=list(range(8)))

    out = np.empty((B, S, D), dtype=np.float32)
    for b in range(B):
        U0 = res.results[2 * b]["O"]
        U1 = res.results[2 * b + 1]["O"]
        r0 = res.results[2 * b]["R"].reshape(S)
        r1 = res.results[2 * b + 1]["R"].reshape(S)
        out[b] = (U0 + U1) / (r0 + r1)[:, None]
    return out
